# revision 36
# baseline (speedup 1.0000x reference)
"""Trainium2 Bass kernel for ragged KeyQueryAttention pooling.

Math (per batch b):
    logits[t] = ||x_t @ U1||^2 - ||x_t @ U2||^2,  U1 = (K+Q)/2, U2 = (K-Q)/2
    att = softmax(logits over valid t);  out[b] = sum_t att[t] * x[t, :] + bias

Device strategy (8 NeuronCores, data-parallel over batch; measured
112.2us vs the 413.4us per-chunk fp32 baseline):
  - B=64 batches sorted by length (desc), dealt round-robin into 8 slots
    per core; slot chunk count n_j = ceil(group_max_len/128) (min 4).
    One SPMD program, value-specialized on n_list; identical per core.
  - Slot data is DMA'd in 4-chunk pieces of a layout where partition p
    holds DRAM rows [p*n, (p+1)*n); chunk c = {row p*n + c}_p. The row
    permutation along T is math-invariant; masks follow the mapping.
    Per-partition lines are n*512B contiguous, so DMA runs at full rate.
  - float32r on the PE everywhere: 1 cycle/row at >=256 moving columns
    (vs 4 for fp32), fp32 bit layout with an 11-bit mantissa (end-to-end
    rel err 1.5e-3). Inputs stream from DMA as f32r; SBUF-produced f32r
    operands come from ScalarE activations (DVE may only move f32r) .
  - Per 4-chunk group: 4 PE transposes (x chunk stationary, f32r
    identity moving, 86ns each) -> xT [128,512] PSUM; ScalarE/DVE copy
    -> SBUF f32r; one proj matmul kq (stationary) x xT (moving 512) ->
    G^T [2L,512] PSUM; ScalarE Square -> Gsq f32r SBUF; one sign-matmul
    (one-hot +/-1 banded stationary) accumulating logits^T row g of a
    per-slot [G,512] PSUM stack. Groups are phase-batched in packets of
    12 (all transposes, then projs with signs interleaved 2 behind,
    hiding the Square latency) to keep the PE
    stream homogeneous - f32r weight loads are fused+serial, so long
    same-shape runs minimize LDW stalls.
  - Per slot: one DVE copy of the logits stack, 4 tiny PE transposes ->
    logits columns [128, 4G]; DVE mask-add (-1e30 pads); DVE rowmax;
    GpSimd partition all-reduce(max); DVE negate; ScalarE exp
    (bias=-max, accum_out=zrow column).
  - Weighted sum: per group one f32r matmul att4 (stationary [128,4]) x
    X4 (moving [128,<=512]) accumulated into a per-slot [4,512] PSUM
    tile; diagonal block m holds sum_t att*x over chunks c = m (mod 4).
    Host sums the 4 diagonal blocks, divides by z = sum(zrow), adds
    bias. Off-diagonal cross terms are simply ignored.
  - Slot tails (softmax + weighted sum) are emitted inside the next
    slot's first packet so the PE never stalls on the softmax latency
    chain; slots run largest-first so the final tail is the smallest.
"""

import os
import numpy as np

import concourse.bass as bass
import concourse.bacc as bacc
import concourse.tile as tile
from concourse import mybir
from concourse import bass_isa
from concourse.bass_utils import run_bass_kernel_spmd

B, T, D, L = 64, 8192, 128, 64
NCORES = 8
SLOTS = B // NCORES  # 8 slots per core
F32 = mybir.dt.float32
F32R = mybir.dt.float32r
AF = mybir.ActivationFunctionType
ALU = mybir.AluOpType

LAST_EXEC_NS = None  # filled when KQA_TRACE=1

_PROG_CACHE = {}


def _build_program(n_list):
    nc = bacc.Bacc()
    ntot = sum(4 * (-(-n // 4)) for n in n_list)
    xs = [
        nc.declare_dram_parameter(f"x{j}", [128, n * D], F32R, isOutput=False)
        for j, n in enumerate(n_list)
    ]
    kq = nc.declare_dram_parameter("kq", [D, 2 * L], F32R, isOutput=False)
    idr = nc.declare_dram_parameter("idr", [128, 128], F32R, isOutput=False)
    id16 = nc.declare_dram_parameter("id16", [16, 16], F32, isOutput=False)
    sgnb = nc.declare_dram_parameter("sgnb", [128, 31], F32R, isOutput=False)
    maskp = nc.declare_dram_parameter("mask", [128, ntot], F32, isOutput=False)
    outw = nc.declare_dram_parameter("outw", [4, SLOTS * 512], F32, isOutput=True)
    outz = nc.declare_dram_parameter("outz", [128, SLOTS], F32, isOutput=True)

    with tile.TileContext(nc) as tc:
        with (
            tc.tile_pool(name="consts", bufs=1) as consts,
            tc.tile_pool(name="xpool", bufs=3) as xpool,
            tc.tile_pool(name="spool", bufs=3) as spool,
            tc.tile_pool(name="tpool", bufs=14) as tpool,
            tc.tile_pool(name="psT", bufs=2, space="PSUM") as psT,
            tc.tile_pool(name="psG", bufs=3, space="PSUM") as psG,
            tc.tile_pool(name="psL", bufs=1, space="PSUM") as psL,
            tc.tile_pool(name="psC", bufs=1, space="PSUM") as psC,
            tc.tile_pool(name="psW", bufs=1, space="PSUM") as psW,
        ):
            identr = consts.tile([128, 128], F32R)
            nc.scalar.dma_start(out=identr, in_=idr[:, :])
            kq_sb = consts.tile([D, 2 * L], F32R)
            nc.scalar.dma_start(out=kq_sb, in_=kq[:, :])
            sgn_sb = consts.tile([128, 31], F32R)
            nc.scalar.dma_start(out=sgn_sb, in_=sgnb[:, :])
            ident16 = consts.tile([16, 16], F32)
            nc.scalar.dma_start(out=ident16, in_=id16[:, :])
            mask_sb = consts.tile([128, ntot], F32)
            nc.scalar.dma_start(out=mask_sb, in_=maskp[:, :])
            wacc_sb = consts.tile([4, SLOTS * 512], F32)
            z_sb = consts.tile([128, SLOTS], F32)

            def make_tail(j, n, off, x_sb, lg_sb):
                G = -(-n // 4)
                n4 = 4 * G

                def tail():
                    # logits rows -> columns: 4 tiny transposes
                    lcol_ps = psC.tile([128, G, 4], F32, tag="lcol")
                    for c4 in range(4):
                        nc.tensor.transpose(
                            lcol_ps[:, :, c4],
                            lg_sb[:, c4 * 128 : (c4 + 1) * 128],
                            ident16[0:G, 0:G],
                        )
                    lcol = spool.tile([128, n4], F32, tag="lcol_sb")
                    nc.vector.tensor_tensor(
                        lcol, lcol_ps.rearrange("p a b -> p (a b)"),
                        mask_sb[:, off : off + n4],
                        op=ALU.add,
                    )
                    rmax = spool.tile([128, 1], F32, tag="rmax")
                    nc.vector.tensor_reduce(
                        rmax, lcol, axis=mybir.AxisListType.X, op=ALU.max
                    )
                    amax = spool.tile([128, 1], F32, tag="amax")
                    nc.gpsimd.partition_all_reduce(
                        amax, rmax, channels=128,
                        reduce_op=bass_isa.ReduceOp.max,
                    )
                    negm = spool.tile([128, 1], F32, tag="negm")
                    nc.vector.tensor_scalar_mul(negm, amax, -1.0)
                    p_sb = spool.tile([128, n4], F32R, tag="p")
                    nc.scalar.activation(
                        p_sb, lcol, AF.Exp, bias=negm, scale=1.0,
                        accum_out=z_sb[:, j : j + 1],
                    )
                    # weighted sum: cross-product accumulation, diag wanted
                    wacc_ps = psW.tile([4, 512], F32, tag="wacc")
                    for g in range(G):
                        w = min(4, n - 4 * g)
                        nc.tensor.matmul(
                            wacc_ps[:, : w * 128],
                            p_sb[:, 4 * g : 4 * g + 4],
                            x_sb[:, 4 * g : 4 * g + w, :],
                            start=(g == 0),
                            stop=(g == G - 1),
                            skip_group_check=True,
                        )
                    nc.vector.tensor_copy(
                        wacc_sb[:, j * 512 : (j + 1) * 512], wacc_ps
                    )

                return tail

            G_list = [-(-n // 4) for n in n_list]
            offs = np.concatenate([[0], np.cumsum([4 * G for G in G_list])]).astype(int)
            PKT = 12  # groups per phase-batched packet

            pending = None
            for j in range(SLOTS):
                n = n_list[j]
                off = int(offs[j])
                G = G_list[j]
                x_sb = xpool.tile([128, n, D], F32R, tag="x")
                for g in range(G):
                    w = min(4, n - 4 * g)
                    nc.sync.dma_start(
                        out=x_sb[:, 4 * g : 4 * g + w, :],
                        in_=xs[j][:, g * 512 : g * 512 + w * 128],
                    )
                lg_ps = psL.tile([G, 512], F32, tag="lg")

                all_gsqs = {}

                def sign_mm(g):
                    w = min(4, n - 4 * g)
                    nc.tensor.matmul(
                        lg_ps[:, : w * 128],
                        sgn_sb[:, 15 - g : 15 - g + G],
                        all_gsqs.pop(g)[:, : w * 128],
                        start=(g == 0),
                        stop=(g == G - 1),
                        skip_group_check=True,
                    )

                pend_signs = []
                for p0 in range(0, G, PKT):
                    p1 = min(p0 + PKT, G)
                    xts = []
                    for g in range(p0, p1):
                        w = min(4, n - 4 * g)
                        xT_ps = psT.tile([128, 512], F32R, tag="xT")
                        for c4 in range(w):
                            nc.tensor.transpose(
                                xT_ps[:, c4 * 128 : (c4 + 1) * 128],
                                x_sb[:, 4 * g + c4, :],
                                identr,
                            )
                        xT_sb = tpool.tile([128, 512], F32R, tag="xTs")
                        if g % 4 == 0:
                            nc.scalar.activation(
                                xT_sb[:, : w * 128],
                                xT_ps[:, : w * 128].bitcast(F32), AF.Copy
                            )
                        else:
                            nc.vector.tensor_copy(
                                xT_sb[:, : w * 128], xT_ps[:, : w * 128]
                            )
                        xts.append(xT_sb)
                        # drain signs carried over from the previous packet
                        if pend_signs and g - p0 >= 1:
                            sign_mm(pend_signs.pop(0))
                    for gs in pend_signs:
                        sign_mm(gs)
                    pend_signs = []
                    for g in range(p0, p1):
                        w = min(4, n - 4 * g)
                        gt_ps = psG.tile([128, 512], F32, tag="gt")
                        nc.tensor.matmul(
                            gt_ps[:, : w * 128], kq_sb,
                            xts[g - p0][:, : w * 128], start=True, stop=True
                        )
                        gsq_sb = tpool.tile([128, 512], F32R, tag="gsq")
                        nc.scalar.activation(
                            gsq_sb[:, : w * 128], gt_ps[:, : w * 128],
                            AF.Square
                        )
                        all_gsqs[g] = gsq_sb
                        if g - 2 >= p0:
                            sign_mm(g - 2)
                    pend_signs = [g for g in range(max(p0, p1 - 2), p1)]
                    if p1 == G:
                        for gs in pend_signs:
                            sign_mm(gs)
                        pend_signs = []
                    if p0 == 0 and pending is not None:
                        pending()
                        pending = None
                        if j == SLOTS - 1:
                            nc.sync.dma_start(
                                out=outw[:, : (SLOTS - 1) * 512],
                                in_=wacc_sb[:, : (SLOTS - 1) * 512],
                            )
                            nc.sync.dma_start(
                                out=outz[:, : SLOTS - 1],
                                in_=z_sb[:, : SLOTS - 1],
                            )
                lg_sb = spool.tile([G, 512], F32, tag="lgs")
                nc.vector.tensor_copy(lg_sb, lg_ps)
                pending = make_tail(j, n, off, x_sb, lg_sb)
            pending()
            nc.sync.dma_start(
                out=outw[:, (SLOTS - 1) * 512 :],
                in_=wacc_sb[:, (SLOTS - 1) * 512 :],
            )
            nc.sync.dma_start(
                out=outz[:, SLOTS - 1 :], in_=z_sb[:, SLOTS - 1 :]
            )
    nc.finalize()
    return nc


def kernel(seq, lengths, key_w, query_w, bias):
    global LAST_EXEC_NS
    seq = np.ascontiguousarray(np.asarray(seq, dtype=np.float32))
    lengths_np = np.asarray(lengths).astype(np.int64)
    key_w = np.asarray(key_w, dtype=np.float32)
    query_w = np.asarray(query_w, dtype=np.float32)
    bias = np.asarray(bias, dtype=np.float32)

    order = np.argsort(-lengths_np, kind="stable")  # descending length
    n_list = []
    for j in range(SLOTS):
        grp = order[j * NCORES : (j + 1) * NCORES]
        n = max(4, int(-(-int(lengths_np[grp].max()) // 128)))
        n_list.append(n)
    key = tuple(n_list)
    if key not in _PROG_CACHE:
        _PROG_CACHE[key] = _build_program(n_list)
    nc = _PROG_CACHE[key]

    def rnd(a):  # fp32r host rounding for small constants
        return (a.astype(np.float32).view(np.uint32) & np.uint32(0xFFFFF000)).view(
            np.float32
        )

    u1 = (key_w + query_w) * 0.5
    u2 = (key_w - query_w) * 0.5
    kqcat = rnd(np.concatenate([u1, u2], axis=1))  # [D, 2L]
    sgn = np.concatenate([np.ones(L), -np.ones(L)]).astype(np.float32)
    sgnb = np.zeros((128, 31), dtype=np.float32)
    sgnb[:, 15] = sgn
    idr = np.eye(128, dtype=np.float32)
    id16 = np.eye(16, dtype=np.float32)

    in_maps = []
    for i in range(NCORES):
        m = {"kq": kqcat, "idr": idr, "id16": id16, "sgnb": sgnb}
        mask_cols = []
        for j, n in enumerate(n_list):
            b = int(order[j * NCORES + i])
            m[f"x{j}"] = seq[b, : n * 128, :].reshape(128, n * D)
            lb = int(lengths_np[b])
            n4 = 4 * (-(-n // 4))
            # row at (partition p, chunk c) is p*n + c; chunks >= n padded
            rows = np.arange(128)[:, None] * n + np.arange(n4)[None, :]
            valid = (rows < lb) & (np.arange(n4)[None, :] < n)
            col = np.where(valid, 0.0, -1e30).astype(np.float32)
            mask_cols.append(col)
        m["mask"] = np.ascontiguousarray(np.concatenate(mask_cols, axis=1))
        in_maps.append(m)

    trace = os.environ.get("KQA_TRACE") == "1"
    res = run_bass_kernel_spmd(
        nc, in_maps, core_ids=list(range(NCORES)), trace=trace
    )
    LAST_EXEC_NS = res.exec_time_ns

    out = np.empty((B, D), dtype=np.float32)
    for i in range(NCORES):
        w = res.results[i]["outw"].reshape(4, SLOTS, 512)
        zr = res.results[i]["outz"]  # [128, SLOTS]
        for j in range(SLOTS):
            b = int(order[j * NCORES + i])
            acc = np.zeros(D, dtype=np.float64)
            for c4 in range(4):
                acc += w[c4, j, c4 * 128 : (c4 + 1) * 128].astype(np.float64)
            z = zr[:, j].astype(np.float64).sum()
            out[b] = (acc / z + bias.astype(np.float64)).astype(np.float32)
    return out
